# revision 3
# baseline (speedup 1.0000x reference)
"""BoundaryLoss TRN2 kernel — 8-core data-parallel (b x H-half).

Math (exact restructuring of the reference):
  p = sigmoid(inputs); mask_p = (p != 0) = 1 everywhere for this data regime
  (|logits| < 40), so erode6(mask_p) = E = interior indicator (0 on any
  volume face, 1 inside). boundary_inputs = p0 + p1 - 2E.
  Interior voxels: p0+p1-2 < 0  =>  bi = clip(.) = EPS exactly, so the
  per-voxel loss is affine in bt = boundary_targets:
      f_int(bt) = -(bt*log(EPS) + (1-bt)*log1p(-EPS))
  Face voxels (d in {0,127} or h in {0,191} or w in {0,191}):
      bi = clip(p0+p1, EPS, 1-EPS),  bt = t0 + t1  (erosion of targets is 0
      at faces), full BCE evaluated directly.
  Total = sum_int f_int(bt) + sum_faces f(bt, bi); the only dense device
  work is the 6-connectivity erosion of the two target channels and exact
  popcount-style sums of the boundary map.

Device pipeline per core (b, H-half), SPMD on 8 NeuronCores:
  - targets slab int32 [2, 128, 98*192] (1-row halos, zeros at volume edge)
    DMA-cast to int8; u = t0 | (t1 << 3) packs both channels per byte.
  - erosion via pure-bitwise AND of 7 taps (w+-1: byte-shifted SBUF-SBUF DMA
    copies; d+-1: partition-shifted DMA copies; h+-1: in-tile views).
  - B = u ^ e  -> bytes bt0 + 8*bt1.
  - Sums via ScalarE activation(Copy) accum_out (fp32-exact integer sums).
  - Small host-gathered face arrays get the full BCE on device.
"""
import sys
sys.path.insert(0, "/opt/trn_rl_repo")

import numpy as np

B_DIM, C_DIM, D_DIM, H_DIM, W_DIM = 4, 2, 128, 192, 192
N_CORES = 8
HH = H_DIM // 2            # 96 own rows per core
SLAB_ROWS = HH + 2         # with halo
ROW_B = W_DIM              # 192 bytes per row (int8)
CHUNK_ROWS = 32            # own rows per chunk
N_CHUNKS = HH // CHUNK_ROWS
OWN_B = CHUNK_ROWS * ROW_B           # 6144 bytes per chunk (own window)
OWN_W = OWN_B // 4                   # 1536 int32 words
LOAD_ROWS = CHUNK_ROWS + 2           # 34
LOAD_B = LOAD_ROWS * ROW_B           # 6528
FACE_N = 2 * HH * W_DIM + (D_DIM - 2) * W_DIM + (D_DIM - 2) * (HH - 1) * 2  # 84996
FACE_F = 672                         # per-partition face elems (128*672 = 86016)
FACE_PAD = 128 * FACE_F - FACE_N
EPS = 1e-7
N_MEAN = B_DIM * D_DIM * H_DIM * W_DIM  # 18874368
OUT_COLS = 16

_compiled = None


def _build_bass():
    import concourse.bacc as bacc
    import concourse.tile as tile
    from concourse import mybir
    from contextlib import ExitStack

    dt = mybir.dt
    Alu = mybir.AluOpType
    P = 128

    nc = bacc.Bacc("TRN2", target_bir_lowering=False, debug=False,
                   num_devices=N_CORES)
    tslab = nc.declare_dram_parameter(
        "tslab", [C_DIM, P, SLAB_ROWS * ROW_B], dt.int32, isOutput=False)
    xf = nc.declare_dram_parameter(
        "xf", [C_DIM, P, FACE_F], dt.float32, isOutput=False)
    btf = nc.declare_dram_parameter(
        "btf", [P, FACE_F], dt.float32, isOutput=False)
    out = nc.declare_dram_parameter(
        "out", [P, OUT_COLS], dt.float32, isOutput=True)

    import os as _os
    _bufs = int(_os.environ.get("BDL_BUFS", "2"))
    _repeat = int(_os.environ.get("BDL_REPEAT", "1"))
    with tile.TileContext(nc) as tc, ExitStack() as ctx:
        io_pool = ctx.enter_context(tc.tile_pool(name="io", bufs=_bufs))
        sh_pool = ctx.enter_context(tc.tile_pool(name="sh", bufs=_bufs))
        small = ctx.enter_context(tc.tile_pool(name="small", bufs=1))

        sc3 = small.tile([P, 1], dt.int32)
        nc.vector.memset(sc3[:], 3)
        zrow = small.tile([1, OWN_B], dt.int8)
        nc.vector.memset(zrow[:], 0)

        stage = small.tile([P, OUT_COLS], dt.float32)
        nc.vector.memset(stage[:], 0.0)

        for ck in [c for _ in range(_repeat) for c in range(N_CHUNKS)]:
            r0 = ck * CHUNK_ROWS           # slab row of chunk halo start
            lo_b = r0 * ROW_B              # load window byte offset

            q0 = io_pool.tile([P, LOAD_B], dt.int8, tag="q0")
            q1 = io_pool.tile([P, LOAD_B], dt.int8, tag="q1")
            nc.gpsimd.dma_start(q0[:], tslab[0, :, lo_b:lo_b + LOAD_B])
            nc.gpsimd.dma_start(q1[:], tslab[1, :, lo_b:lo_b + LOAD_B])

            u = io_pool.tile([P, LOAD_B], dt.int8, tag="u")
            uw = u[:].bitcast(dt.int32)
            nc.vector.scalar_tensor_tensor(
                uw, q1[:].bitcast(dt.int32), sc3[:, 0:1], q0[:].bitcast(dt.int32),
                op0=Alu.logical_shift_left, op1=Alu.bitwise_or)

            # shifted copies of the own window (bytes [192, 6336))
            uw1 = sh_pool.tile([P, OWN_B], dt.int8, tag="uw1")
            uwm1 = sh_pool.tile([P, OWN_B], dt.int8, tag="uwm1")
            ud1 = sh_pool.tile([P, OWN_B], dt.int8, tag="ud1")
            udm1 = sh_pool.tile([P, OWN_B], dt.int8, tag="udm1")
            nc.sync.dma_start(uw1[:], u[:, ROW_B - 1:ROW_B - 1 + OWN_B])
            nc.sync.dma_start(uwm1[:], u[:, ROW_B + 1:ROW_B + 1 + OWN_B])
            nc.sync.dma_start(ud1[0:P - 1, :], u[1:P, ROW_B:ROW_B + OWN_B])
            nc.sync.dma_start(udm1[1:P, :], u[0:P - 1, ROW_B:ROW_B + OWN_B])
            # zero out-of-volume taps
            uw1_3d = uw1[:].rearrange("p (r w) -> p r w", w=ROW_B)
            uwm1_3d = uwm1[:].rearrange("p (r w) -> p r w", w=ROW_B)
            nc.vector.memset(uw1_3d[:, :, 0:1], 0)
            nc.vector.memset(uwm1_3d[:, :, ROW_B - 1:ROW_B], 0)
            nc.sync.dma_start(ud1[P - 1:P, :], zrow[:])
            nc.sync.dma_start(udm1[0:1, :], zrow[:])

            # erosion: e = uo & all 6 neighbor taps (pure bitwise, exact)
            uo = uw[:, 48:48 + OWN_W]              # own window (words)
            uh1 = uw[:, 96:96 + OWN_W]             # h+1 view
            uhm1 = uw[:, 0:OWN_W]                  # h-1 view
            e_t = sh_pool.tile([P, OWN_B], dt.int8, tag="e")
            ew = e_t[:].bitcast(dt.int32)
            nc.vector.tensor_tensor(ew, uo, uh1, op=Alu.bitwise_and)
            nc.vector.tensor_tensor(ew, ew, uhm1, op=Alu.bitwise_and)
            nc.vector.tensor_tensor(ew, ew, uw1[:].bitcast(dt.int32), op=Alu.bitwise_and)
            nc.vector.tensor_tensor(ew, ew, uwm1[:].bitcast(dt.int32), op=Alu.bitwise_and)
            nc.vector.tensor_tensor(ew, ew, ud1[:].bitcast(dt.int32), op=Alu.bitwise_and)
            nc.vector.tensor_tensor(ew, ew, udm1[:].bitcast(dt.int32), op=Alu.bitwise_and)

            # B = u ^ e : bytes = bt0 + 8*bt1
            B_t = sh_pool.tile([P, OWN_B], dt.int8, tag="B")
            Bw = B_t[:].bitcast(dt.int32)
            nc.vector.tensor_tensor(Bw, uo, ew, op=Alu.bitwise_xor)

            # sums: col ck = sum(B bytes) = Sbt0 + 8*Sbt1 ; col 3+ck = Sbt1
            m1 = sh_pool.tile([P, OWN_B], dt.int8, tag="m1")
            nc.vector.tensor_scalar(
                m1[:].bitcast(dt.int32), Bw, 3, 0x01010101,
                op0=Alu.logical_shift_right, op1=Alu.bitwise_and)
            junk = sh_pool.tile([P, OWN_B], dt.int8, tag="junk")
            accB = small.tile([P, 1], dt.float32, tag=f"accB{ck}")
            acc1 = small.tile([P, 1], dt.float32, tag=f"acc1{ck}")
            nc.scalar.activation(junk[:], B_t[:],
                                 mybir.ActivationFunctionType.Copy,
                                 accum_out=accB[:])
            nc.scalar.activation(junk[:], m1[:],
                                 mybir.ActivationFunctionType.Copy,
                                 accum_out=acc1[:])
            nc.vector.tensor_copy(stage[:, ck:ck + 1], accB[:])
            nc.vector.tensor_copy(stage[:, 3 + ck:4 + ck], acc1[:])

        # ---- face BCE pass ----
        _variant = _os.environ.get("BDL_VARIANT", "full")
        for _rep in range(_repeat):
            xf0 = small.tile([P, FACE_F], dt.float32, tag="xf0")
            xf1 = small.tile([P, FACE_F], dt.float32, tag="xf1")
            btft = small.tile([P, FACE_F], dt.float32, tag="btft")
            nc.sync.dma_start(xf0[:], xf[0])
            nc.sync.dma_start(xf1[:], xf[1])
            nc.sync.dma_start(btft[:], btf[:])

            if _variant != "noface":
                s0 = small.tile([P, FACE_F], dt.float32, tag="s0")
                s1 = small.tile([P, FACE_F], dt.float32, tag="s1")
                nc.scalar.activation(s0[:], xf0[:], mybir.ActivationFunctionType.Sigmoid)
                nc.scalar.activation(s1[:], xf1[:], mybir.ActivationFunctionType.Sigmoid)
                ps = small.tile([P, FACE_F], dt.float32, tag="ps")
                nc.vector.tensor_tensor(ps[:], s0[:], s1[:], op=Alu.add)
                bi = small.tile([P, FACE_F], dt.float32, tag="bi")
                nc.vector.tensor_scalar(bi[:], ps[:], float(EPS), float(1.0 - EPS),
                                        op0=Alu.max, op1=Alu.min)
                lg1 = small.tile([P, FACE_F], dt.float32, tag="lg1")
                lg2 = small.tile([P, FACE_F], dt.float32, tag="lg2")
                nc.scalar.activation(lg1[:], bi[:], mybir.ActivationFunctionType.Ln)
                nc.scalar.activation(lg2[:], bi[:], mybir.ActivationFunctionType.Ln,
                                     scale=-1.0, bias=1.0)
                dlg = small.tile([P, FACE_F], dt.float32, tag="dlg")
                nc.vector.tensor_tensor(dlg[:], lg1[:], lg2[:], op=Alu.subtract)
                m_t = small.tile([P, FACE_F], dt.float32, tag="m_t")
                nc.vector.tensor_tensor(m_t[:], btft[:], dlg[:], op=Alu.mult)
                fsum = small.tile([P, FACE_F], dt.float32, tag="fsum")
                facc = small.tile([P, 1], dt.float32, tag="facc")
                nc.vector.tensor_tensor(fsum[:], m_t[:], lg2[:], op=Alu.add)
                nc.vector.tensor_reduce(facc[:], fsum[:],
                                        axis=mybir.AxisListType.X, op=Alu.add)
                btacc = small.tile([P, 1], dt.float32, tag="btacc")
                nc.vector.tensor_reduce(btacc[:], btft[:], axis=mybir.AxisListType.X,
                                        op=Alu.add)
                nc.vector.tensor_copy(stage[:, 6:7], btacc[:])
                nc.vector.tensor_copy(stage[:, 7:8], facc[:])
            else:
                btacc = small.tile([P, 1], dt.float32, tag="btacc")
                nc.vector.tensor_reduce(btacc[:], btft[:], axis=mybir.AxisListType.X,
                                        op=Alu.add)
                nc.vector.tensor_copy(stage[:, 6:7], btacc[:])

        nc.sync.dma_start(out[:], stage[:])

    nc.compile()
    return nc


def _face_indices(half):
    """Flat voxel indices (into a [128,192,192] volume) for this H-half's
    deduped face set, in canonical order. Same for every b."""
    h0 = HH * half
    h_edge = 0 if half == 0 else H_DIM - 1
    own_h = np.arange(h0, h0 + HH)
    idx = []
    # F1: d in {0,127} x own h x all w
    for d in (0, D_DIM - 1):
        ii = (d * H_DIM + own_h)[:, None] * W_DIM + np.arange(W_DIM)[None, :]
        idx.append(ii.ravel())
    # F2: h = h_edge, d in [1,126], all w
    dd = np.arange(1, D_DIM - 1)
    ii = (dd * H_DIM + h_edge)[:, None] * W_DIM + np.arange(W_DIM)[None, :]
    idx.append(ii.ravel())
    # F3: d in [1,126], own h minus h_edge, w in {0,191}
    hs = own_h[own_h != h_edge]
    ii = ((dd[:, None] * H_DIM + hs[None, :])[:, :, None] * W_DIM
          + np.array([0, W_DIM - 1])[None, None, :])
    idx.append(ii.ravel())
    idx = np.concatenate(idx)
    assert idx.size == FACE_N
    return idx


def _stage_inputs(inputs, targets):
    """Build per-core input dicts."""
    face_idx = [_face_indices(0), _face_indices(1)]
    in_maps = []
    tg = np.ascontiguousarray(targets)
    xg = np.ascontiguousarray(inputs)
    for core in range(N_CORES):
        b, half = divmod(core, 2)
        h0 = HH * half
        slab = np.zeros((C_DIM, D_DIM, SLAB_ROWS, W_DIM), dtype=np.int32)
        lo = max(h0 - 1, 0)
        hi = min(h0 + HH + 1, H_DIM)
        slab[:, :, lo - (h0 - 1):lo - (h0 - 1) + (hi - lo), :] = \
            tg[b, :, :, lo:hi, :]
        slab = slab.reshape(C_DIM, D_DIM, SLAB_ROWS * W_DIM)

        fi = face_idx[half]
        xf = np.full((C_DIM, 128 * FACE_F), -40.0, dtype=np.float32)
        btf = np.zeros((128 * FACE_F,), dtype=np.float32)
        for c in range(C_DIM):
            xf[c, :FACE_N] = xg[b, c].reshape(-1)[fi]
        tflat0 = tg[b, 0].reshape(-1)[fi]
        tflat1 = tg[b, 1].reshape(-1)[fi]
        btf[:FACE_N] = (tflat0 + tflat1).astype(np.float32)
        in_maps.append({
            "tslab": slab,
            "xf": xf.reshape(C_DIM, 128, FACE_F),
            "btf": btf.reshape(128, FACE_F),
        })
    return in_maps


def _combine(results):
    """Host-side exact combination of per-core partials (float64)."""
    Leps = float(np.log(np.float32(EPS)))
    L1m = float(np.log1p(np.float32(-EPS)))
    n_int_core = 128 * HH * W_DIM - FACE_N
    total = 0.0
    for r in results:
        o = r["out"].astype(np.float64)
        sB = o[:, 0:3].sum()
        s1 = o[:, 3:6].sum()
        sbt1 = s1
        sbt0 = sB - 8.0 * sbt1
        sbt_all = sbt0 + sbt1
        sbt_face = o[:, 6].sum()
        face_raw = o[:, 7].sum()
        interior = n_int_core * (-L1m) + (L1m - Leps) * (sbt_all - sbt_face)
        total += interior + (-face_raw)
    return total / N_MEAN


def _get_compiled():
    global _compiled
    if _compiled is None:
        _compiled = _build_bass()
    return _compiled


def kernel(inputs, targets):
    from concourse.bass_utils import run_bass_kernel_spmd
    nc = _get_compiled()
    in_maps = _stage_inputs(np.asarray(inputs), np.asarray(targets))
    res = run_bass_kernel_spmd(nc, in_maps, list(range(N_CORES)))
    mean = _combine(res.results)
    return np.float32(mean)



# revision 7
# speedup vs baseline: 4.7543x; 4.7543x over previous
"""BoundaryLoss TRN2 kernel — 8-core data-parallel (b x H-half), bit-packed.

Math (exact restructuring of the reference):
  p = sigmoid(inputs); mask_p = (p != 0) = 1 everywhere for this data regime
  (|logits| < 40), so erode6(mask_p) = E = interior indicator (0 on any
  volume face, 1 inside). boundary_inputs = p0 + p1 - 2E.
  Interior voxels: p0+p1-2 < 0  =>  bi = clip(.) = EPS exactly, so the
  per-voxel loss is affine in bt = boundary_targets:
      f_int(bt) = -(bt*log(EPS) + (1-bt)*log1p(-EPS))
  and the dense reduction only needs S01 = sum(bt0 + bt1) over the volume.
  Face voxels (d in {0,127} or h in {0,191} or w in {0,191}):
      bi = clip(p0+p1, EPS, 1-EPS), bt = t0 + t1; full BCE on gathered faces.

Device pipeline per core (b, H-half), SPMD on 8 NeuronCores:
  - targets bit-packed on host: 2 bits/voxel (t0 at bit 2i, t1 at bit 2i+1,
    16 voxels per int32 word along W), rows of 12 data words + 1 zero pad
    word so the w+-1 shift carries are zero at row edges. Slab [128, 1280]
    int32 per core (partition = D, free = 98 H-rows incl 1-row halos).
  - erosion = AND of 7 taps: w+-1 via (v<<2 | carry)/(v>>2 | carry) with
    word-offset views, h+-1 via +-13-word views, d+-1 via partition-shifted
    DMA copies. B = v ^ e has per-voxel 2-bit boundary flags.
  - popcount(B) via SWAR (5/3/0F nibble ladder) to bytes, summed exactly by
    ScalarE activation(Copy) accum_out in fp32.
  - Small host-gathered face arrays get the full BCE on device.
"""
import sys
sys.path.insert(0, "/opt/trn_rl_repo")

import os as _os
import numpy as np

B_DIM, C_DIM, D_DIM, H_DIM, W_DIM = 4, 2, 128, 192, 192
N_CORES = 8
HH = H_DIM // 2            # 96 own rows per core
SLAB_ROWS = HH + 2         # with halo
ROW_W = 13                 # 12 data words + 1 zero pad word per row
ROW_DATA_B = 48            # 192 voxels * 2 bits = 48 bytes
ROW_B = ROW_W * 4          # 52 bytes per row
SLAB_W = 1280              # 1 lead pad + 98*13 = 1275 words, rounded up
OWN_OFF = 1 + ROW_W        # own window starts at row 1
OWN_W = HH * ROW_W         # 1248 words
FACE_N = 2 * HH * W_DIM + (D_DIM - 2) * W_DIM + (D_DIM - 2) * (HH - 1) * 2  # 84996
FACE_F = 672                         # per-partition face elems (128*672 = 86016)
EPS = 1e-7
N_MEAN = B_DIM * D_DIM * H_DIM * W_DIM  # 18874368
OUT_COLS = 16

_compiled = None


def _build_bass():
    import concourse.bacc as bacc
    import concourse.tile as tile
    from concourse import mybir
    from contextlib import ExitStack

    dt = mybir.dt
    Alu = mybir.AluOpType
    Act = mybir.ActivationFunctionType
    P = 128

    nc = bacc.Bacc("TRN2", target_bir_lowering=False, debug=False,
                   num_devices=N_CORES)
    vslab = nc.declare_dram_parameter(
        "vslab", [P, SLAB_W], dt.int32, isOutput=False)
    xf = nc.declare_dram_parameter(
        "xf", [C_DIM, P, FACE_F], dt.float32, isOutput=False)
    btf = nc.declare_dram_parameter(
        "btf", [P, FACE_F], dt.float32, isOutput=False)
    out = nc.declare_dram_parameter(
        "out", [P, OUT_COLS], dt.float32, isOutput=True)

    _repeat = int(_os.environ.get("BDL_REPEAT", "1"))
    with tile.TileContext(nc) as tc, ExitStack() as ctx:
        pool = ctx.enter_context(tc.tile_pool(name="main", bufs=1))
        small = ctx.enter_context(tc.tile_pool(name="small", bufs=1))

        sc30 = small.tile([P, 1], dt.int32)
        nc.vector.memset(sc30[:], 30)
        zrow = small.tile([1, OWN_W], dt.int32)
        nc.vector.memset(zrow[:], 0)
        stage = small.tile([P, OUT_COLS], dt.float32)
        nc.vector.memset(stage[:], 0.0)

        for _rep in range(_repeat):
            v = pool.tile([P, SLAB_W], dt.int32, tag="v")
            nc.gpsimd.dma_start(v[:], vslab[:])
            vo = v[:, OWN_OFF:OWN_OFF + OWN_W]
            vh1 = v[:, OWN_OFF + ROW_W:OWN_OFF + ROW_W + OWN_W]
            vhm1 = v[:, OWN_OFF - ROW_W:OWN_OFF - ROW_W + OWN_W]
            vprev = v[:, OWN_OFF - 1:OWN_OFF - 1 + OWN_W]
            vnext = v[:, OWN_OFF + 1:OWN_OFF + 1 + OWN_W]

            ud1 = pool.tile([P, OWN_W], dt.int32, tag="ud1")
            udm1 = pool.tile([P, OWN_W], dt.int32, tag="udm1")
            nc.sync.dma_start(ud1[0:P - 1, :], v[1:P, OWN_OFF:OWN_OFF + OWN_W])
            nc.sync.dma_start(ud1[P - 1:P, :], zrow[:])
            nc.sync.dma_start(udm1[1:P, :], v[0:P - 1, OWN_OFF:OWN_OFF + OWN_W])
            nc.sync.dma_start(udm1[0:1, :], zrow[:])

            # w-1 / w+1 taps: 2-bit shifts with cross-word carries; row pads
            # zero the carries at row edges (w=0 / w=191 borders).
            s1 = pool.tile([P, OWN_W], dt.int32, tag="s1")
            nc.vector.tensor_scalar(s1[:], vo, 2, None,
                                    op0=Alu.logical_shift_left)
            wm1 = pool.tile([P, OWN_W], dt.int32, tag="wm1")
            nc.vector.scalar_tensor_tensor(
                wm1[:], vprev, sc30[:, 0:1], s1[:],
                op0=Alu.logical_shift_right, op1=Alu.bitwise_or)
            s2 = pool.tile([P, OWN_W], dt.int32, tag="s2")
            nc.vector.tensor_scalar(s2[:], vo, 2, None,
                                    op0=Alu.logical_shift_right)
            wp1 = pool.tile([P, OWN_W], dt.int32, tag="wp1")
            nc.vector.scalar_tensor_tensor(
                wp1[:], vnext, sc30[:, 0:1], s2[:],
                op0=Alu.logical_shift_left, op1=Alu.bitwise_or)

            # erosion: e = AND of voxel and its 6 face neighbors (per bit)
            e = pool.tile([P, OWN_W], dt.int32, tag="e")
            nc.vector.tensor_tensor(e[:], vo, vh1, op=Alu.bitwise_and)
            nc.vector.tensor_tensor(e[:], e[:], vhm1, op=Alu.bitwise_and)
            nc.vector.tensor_tensor(e[:], e[:], wm1[:], op=Alu.bitwise_and)
            nc.vector.tensor_tensor(e[:], e[:], wp1[:], op=Alu.bitwise_and)
            nc.vector.tensor_tensor(e[:], e[:], ud1[:], op=Alu.bitwise_and)
            nc.vector.tensor_tensor(e[:], e[:], udm1[:], op=Alu.bitwise_and)

            # B = v ^ e (boundary bits), then popcount to bytes. DVE int
            # add/subtract routes through fp32, so large-word SWAR is
            # inexact; instead extract 0x11-masked planes with exact
            # bitwise ops and add them as int8 lanes (every lane <= 68,
            # exact through fp32).
            nc.vector.tensor_tensor(s1[:], vo, e[:], op=Alu.bitwise_xor)
            nc.vector.tensor_scalar(s2[:], s1[:], 0x11111111, None,
                                    op0=Alu.bitwise_and)
            nc.vector.tensor_scalar(wm1[:], s1[:], 1, 0x11111111,
                                    op0=Alu.logical_shift_right,
                                    op1=Alu.bitwise_and)
            nc.vector.tensor_scalar(wp1[:], s1[:], 2, 0x11111111,
                                    op0=Alu.logical_shift_right,
                                    op1=Alu.bitwise_and)
            nc.vector.tensor_scalar(e[:], s1[:], 3, 0x11111111,
                                    op0=Alu.logical_shift_right,
                                    op1=Alu.bitwise_and)
            nc.vector.tensor_tensor(s2[:].bitcast(dt.int8),
                                    s2[:].bitcast(dt.int8),
                                    wm1[:].bitcast(dt.int8), op=Alu.add)
            nc.vector.tensor_tensor(wp1[:].bitcast(dt.int8),
                                    wp1[:].bitcast(dt.int8),
                                    e[:].bitcast(dt.int8), op=Alu.add)
            nc.vector.tensor_tensor(s1[:].bitcast(dt.int8),
                                    s2[:].bitcast(dt.int8),
                                    wp1[:].bitcast(dt.int8), op=Alu.add)
            # s1 bytes = c_low + 16*c_high; fold nibbles to plain counts
            nc.vector.tensor_scalar(s2[:], s1[:], 4, 0x0F0F0F0F,
                                    op0=Alu.logical_shift_right,
                                    op1=Alu.bitwise_and)
            nc.vector.tensor_scalar(wm1[:], s1[:], 0x0F0F0F0F, None,
                                    op0=Alu.bitwise_and)
            nc.vector.tensor_tensor(e[:].bitcast(dt.int8),
                                    s2[:].bitcast(dt.int8),
                                    wm1[:].bitcast(dt.int8), op=Alu.add)

            # exact byte-sum via ScalarE fp32 accumulation
            junk = pool.tile([P, OWN_W * 4], dt.int8, tag="junk")
            acc = small.tile([P, 1], dt.float32, tag="acc")
            nc.scalar.activation(junk[:], e[:].bitcast(dt.int8), Act.Copy,
                                 accum_out=acc[:])
            nc.vector.tensor_copy(stage[:, 0:1], acc[:])

            # ---- face BCE pass ----
            xf0 = small.tile([P, FACE_F], dt.float32, tag="xf0")
            xf1 = small.tile([P, FACE_F], dt.float32, tag="xf1")
            btft = small.tile([P, FACE_F], dt.float32, tag="btft")
            nc.sync.dma_start(xf0[:], xf[0])
            nc.sync.dma_start(xf1[:], xf[1])
            nc.sync.dma_start(btft[:], btf[:])

            s0f = small.tile([P, FACE_F], dt.float32, tag="s0f")
            s1f = small.tile([P, FACE_F], dt.float32, tag="s1f")
            nc.scalar.activation(s0f[:], xf0[:], Act.Sigmoid)
            nc.scalar.activation(s1f[:], xf1[:], Act.Sigmoid)
            ps = small.tile([P, FACE_F], dt.float32, tag="ps")
            nc.vector.tensor_tensor(ps[:], s0f[:], s1f[:], op=Alu.add)
            bi = small.tile([P, FACE_F], dt.float32, tag="bi")
            nc.vector.tensor_scalar(bi[:], ps[:], float(EPS), float(1.0 - EPS),
                                    op0=Alu.max, op1=Alu.min)
            lg1 = small.tile([P, FACE_F], dt.float32, tag="lg1")
            lg2 = small.tile([P, FACE_F], dt.float32, tag="lg2")
            nc.scalar.activation(lg1[:], bi[:], Act.Ln)
            nc.scalar.activation(lg2[:], bi[:], Act.Ln, scale=-1.0, bias=1.0)
            dlg = small.tile([P, FACE_F], dt.float32, tag="dlg")
            nc.vector.tensor_tensor(dlg[:], lg1[:], lg2[:], op=Alu.subtract)
            m_t = small.tile([P, FACE_F], dt.float32, tag="m_t")
            nc.vector.tensor_tensor(m_t[:], btft[:], dlg[:], op=Alu.mult)
            fsum = small.tile([P, FACE_F], dt.float32, tag="fsum")
            facc = small.tile([P, 1], dt.float32, tag="facc")
            nc.vector.tensor_tensor(fsum[:], m_t[:], lg2[:], op=Alu.add)
            nc.vector.tensor_reduce(facc[:], fsum[:],
                                    axis=mybir.AxisListType.X, op=Alu.add)
            btacc = small.tile([P, 1], dt.float32, tag="btacc")
            nc.vector.tensor_reduce(btacc[:], btft[:],
                                    axis=mybir.AxisListType.X, op=Alu.add)
            nc.vector.tensor_copy(stage[:, 6:7], btacc[:])
            nc.vector.tensor_copy(stage[:, 7:8], facc[:])

        nc.sync.dma_start(out[:], stage[:])

    nc.compile()
    return nc


def _face_indices(half):
    """Flat voxel indices (into a [128,192,192] volume) for this H-half's
    deduped face set, in canonical order. Same for every b."""
    h0 = HH * half
    h_edge = 0 if half == 0 else H_DIM - 1
    own_h = np.arange(h0, h0 + HH)
    idx = []
    # F1: d in {0,127} x own h x all w
    for d in (0, D_DIM - 1):
        ii = (d * H_DIM + own_h)[:, None] * W_DIM + np.arange(W_DIM)[None, :]
        idx.append(ii.ravel())
    # F2: h = h_edge, d in [1,126], all w
    dd = np.arange(1, D_DIM - 1)
    ii = (dd * H_DIM + h_edge)[:, None] * W_DIM + np.arange(W_DIM)[None, :]
    idx.append(ii.ravel())
    # F3: d in [1,126], own h minus h_edge, w in {0,191}
    hs = own_h[own_h != h_edge]
    ii = ((dd[:, None] * H_DIM + hs[None, :])[:, :, None] * W_DIM
          + np.array([0, W_DIM - 1])[None, None, :])
    idx.append(ii.ravel())
    idx = np.concatenate(idx)
    assert idx.size == FACE_N
    return idx


def _pack_volume(t0, t1):
    """Pack two binary int channels [D,H,W] into 2-bit interleaved rows of
    13 int32 words (12 data + 1 zero pad): returns uint8 [D, H, 52]."""
    a = (t0 | (t1 << 1)).astype(np.uint8)
    nib = (a[..., 0::4] | (a[..., 1::4] << 2)
           | (a[..., 2::4] << 4) | (a[..., 3::4] << 6))
    vol = np.zeros((D_DIM, H_DIM, ROW_B), dtype=np.uint8)
    vol[..., :ROW_DATA_B] = nib
    return vol


def _stage_inputs(inputs, targets):
    """Build per-core input dicts."""
    face_idx = [_face_indices(0), _face_indices(1)]
    in_maps = []
    tg = np.ascontiguousarray(targets)
    xg = np.ascontiguousarray(inputs)
    vols = [_pack_volume(tg[b, 0], tg[b, 1]) for b in range(B_DIM)]
    for core in range(N_CORES):
        b, half = divmod(core, 2)
        h0 = HH * half
        slab = np.zeros((128, SLAB_W * 4), dtype=np.uint8)
        lo = max(h0 - 1, 0)
        hi = min(h0 + HH + 1, H_DIM)
        rows = np.zeros((128, SLAB_ROWS, ROW_B), dtype=np.uint8)
        rows[:, lo - (h0 - 1):lo - (h0 - 1) + (hi - lo), :] = vols[b][:, lo:hi]
        slab[:, 4:4 + SLAB_ROWS * ROW_B] = rows.reshape(128, -1)
        vslab = slab.view(np.int32)

        fi = face_idx[half]
        xf = np.full((C_DIM, 128 * FACE_F), -40.0, dtype=np.float32)
        btfv = np.zeros((128 * FACE_F,), dtype=np.float32)
        for c in range(C_DIM):
            xf[c, :FACE_N] = xg[b, c].reshape(-1)[fi]
        tflat0 = tg[b, 0].reshape(-1)[fi]
        tflat1 = tg[b, 1].reshape(-1)[fi]
        btfv[:FACE_N] = (tflat0 + tflat1).astype(np.float32)
        in_maps.append({
            "vslab": vslab,
            "xf": xf.reshape(C_DIM, 128, FACE_F),
            "btf": btfv.reshape(128, FACE_F),
        })
    return in_maps


def _combine(results):
    """Host-side exact combination of per-core partials (float64)."""
    Leps = float(np.log(np.float32(EPS)))
    L1m = float(np.log1p(np.float32(-EPS)))
    n_int_core = 128 * HH * W_DIM - FACE_N
    total = 0.0
    for r in results:
        o = r["out"].astype(np.float64)
        s01 = o[:, 0].sum()            # sum(bt0 + bt1) over core's own voxels
        sbt_face = o[:, 6].sum()
        face_raw = o[:, 7].sum()
        interior = n_int_core * (-L1m) + (L1m - Leps) * (s01 - sbt_face)
        total += interior - face_raw
    return total / N_MEAN


def _get_compiled():
    global _compiled
    if _compiled is None:
        _compiled = _build_bass()
    return _compiled


def kernel(inputs, targets):
    from concourse.bass_utils import run_bass_kernel_spmd
    nc = _get_compiled()
    in_maps = _stage_inputs(np.asarray(inputs), np.asarray(targets))
    res = run_bass_kernel_spmd(nc, in_maps, list(range(N_CORES)))
    mean = _combine(res.results)
    return np.float32(mean)
